# revision 15
# baseline (speedup 1.0000x reference)
"""Trainium2 Bass kernel for nn_DeformRouting (deformable routing conv), v3.

Strategy (8 cores, data-parallel over N x H-halves):
  core c handles image n = c//2, row-half = c%2 (14 rows x 28 cols = 392 pixels).

Pipeline (baseline 181us -> 72us):
  - 4-tap bf16 table rows (zero-padded 31x31 grid) -> ONE gather index per
    sample; validity masking folded into table zeros.
  - Per-piece pipeline over 5 gather pieces (9/9/9/5/4 k-slots): gather ->
    combine -> transpose -> column-slice matmuls, so only the small last
    piece's tail is exposed behind the serial SWDGE descriptor generation
    (the dominant cost, ~8ns/idx of gpsimd ucode; ~39us total).
  - Index-permutation matmuls run in bf16 on the SPLIT clipped coords
    (cy, cx <= 30 are bf16-exact); row = 31*cy+cx is fused into the int16
    wrap build.  tile_set_cur_wait paces the scheduler so later pieces'
    combines are not hoisted ahead on the in-order DVE queue.
  - k-major contraction (m = k*64+c, host-permuted weights): contiguous
    bf16 combine writes; W~ and B~ stacked in one [128,128] lhsT chunk so 5
    matmuls per column-slice produce [W~@s ; B~@s] together; out =
    psWB[:64]*x + psWB[64:] per the grouped weight-gen algebra.
"""

import numpy as np

import concourse.bass as bass
import concourse.tile as tile
from concourse import bacc, mybir
from concourse.bass_utils import run_bass_kernel_spmd
from concourse.masks import make_identity

N, CIN, COUT, H, W, K = 4, 64, 64, 28, 28, 3
K2 = K * K
NCORES = 8
HHALF = H // 2          # 14 rows per core
NPT = HHALF * W         # 392 points per core
PCH = 98                # points per partition-chunk
NCH = 4                 # chunks
TBL_ROWS = 31 * 31      # 961 4-tap table rows
SC = (W - 1) / 2.0      # 13.5
NI_CH = 128 * K2        # 1152 gathered rows per chunk
NB = 5                  # m-chunks of 128 (576 -> 640)

F32 = mybir.dt.float32
F32R = mybir.dt.float32r
I32 = mybir.dt.int32
BF16 = mybir.dt.bfloat16

_CACHE = {}


def _alu(name):
    return getattr(mybir.AluOpType, name)


def _build_program():
    nc = bacc.Bacc("TRN2", target_bir_lowering=False, debug=False,
                   num_devices=NCORES)

    tbl4 = nc.dram_tensor("tbl4", [TBL_ROWS, 4 * CIN], BF16, kind="ExternalInput")
    xcpad = nc.dram_tensor("xcpad", [128, NPT], F32, kind="ExternalInput")
    wofft = nc.dram_tensor("wofft", [128, 2 * K2], F32, kind="ExternalInput")
    base2 = nc.dram_tensor("base2", [128, NCH * 2 * K2], F32, kind="ExternalInput")
    wwb = nc.dram_tensor("wwb", [128, NB * 128], BF16, kind="ExternalInput")
    mg = nc.dram_tensor("mg", [128, 8 * 128], BF16, kind="ExternalInput")
    out_d = nc.dram_tensor("out", [COUT, NPT], F32, kind="ExternalOutput")

    mult, add, sub = _alu("mult"), _alu("add"), _alu("subtract")
    is_gt = _alu("is_gt")
    amin, amax = _alu("min"), _alu("max")

    with tile.TileContext(nc) as tc:
        with (
            tc.tile_pool(name="const", bufs=1) as cpool,
            tc.tile_pool(name="work", bufs=1) as wpool,
            tc.tile_pool(name="psoff", bufs=1, space="PSUM") as opool,
            tc.tile_pool(name="pst", bufs=2, space="PSUM") as tpool,
            tc.tile_pool(name="pso", bufs=1, space="PSUM") as popool,
        ):
            # ---- inputs, spread across engine DMA queues ----
            xc_sb = cpool.tile([128, NPT], F32)
            nc.sync.dma_start(xc_sb[:], xcpad.ap())
            wofft_sb = cpool.tile([128, 2 * K2], F32)
            nc.scalar.dma_start(wofft_sb[:], wofft.ap())
            base2_sb = cpool.tile([128, NCH, 2 * K2], F32)
            nc.sync.dma_start(base2_sb[:], base2.ap().rearrange(
                "p (a b) -> p a b", a=NCH))
            wwb_sb = cpool.tile([128, NB, 128], BF16)
            nc.scalar.dma_start(wwb_sb[:], wwb.ap().rearrange(
                "p (a b) -> p a b", a=NB))
            mg_sb = cpool.tile([128, 8, 128], BF16)
            nc.sync.dma_start(mg_sb[:], mg.ap().rearrange(
                "p (a b) -> p a b", a=8))
            identb = cpool.tile([128, 128], BF16)
            make_identity(nc, identb[:])

            # ---- 1. offset conv (pad partitions zeroed: garbage would
            # flow into gather indices past the clip) ----
            ps_off = opool.tile([128, NCH, 2 * K2], F32)
            nc.vector.memset(ps_off[:], 0.0)
            for ch in range(NCH):
                nc.tensor.matmul(
                    out=ps_off[:PCH, ch, :],
                    lhsT=xc_sb[:, ch * PCH:(ch + 1) * PCH],
                    rhs=wofft_sb[:],
                    start=True, stop=True,
                )

            # ---- 2. coordinate math on fused x|y tiles [128, NCH, 18] ----
            shp2 = [128, NCH, 2 * K2]
            _cnt = [0]

            def t(shape=shp2, dt=F32):
                _cnt[0] += 1
                return wpool.tile(shape, dt, name=f"ct{_cnt[0]}")

            ic = t()
            nc.vector.scalar_tensor_tensor(ic[:], ps_off[:], SC, base2_sb[:],
                                           mult, add)
            ti = t(dt=I32)
            nc.any.tensor_copy(ti[:], ic[:])
            tf = t()
            nc.any.tensor_copy(tf[:], ti[:])
            g = t()
            nc.vector.tensor_tensor(g[:], tf[:], ic[:], is_gt)
            f0 = t()
            nc.vector.tensor_tensor(f0[:], tf[:], g[:], sub)
            w1 = t()
            nc.vector.tensor_tensor(w1[:], ic[:], f0[:], sub)
            w0 = t()
            nc.vector.tensor_scalar(w0[:], w1[:], -1.0, 1.0, mult, add)
            cc = t(dt=BF16)
            nc.vector.tensor_scalar(cc[:], f0[:], 30.0, 0.0, amin, amax)
            w4 = t([128, 4, NCH, K2], BF16)
            nc.vector.tensor_tensor(w4[:, 0], w0[:, :, K2:], w0[:, :, :K2], mult)
            nc.vector.tensor_tensor(w4[:, 1], w0[:, :, K2:], w1[:, :, :K2], mult)
            nc.vector.tensor_tensor(w4[:, 2], w1[:, :, K2:], w0[:, :, :K2], mult)
            nc.vector.tensor_tensor(w4[:, 3], w1[:, :, K2:], w1[:, :, :K2], mult)

            # ---- 3. idx wrap: permute clipped coords (bf16-exact ints),
            # then fuse row = 31*cy + cx into the int16 wrap build ----
            psw = opool.tile([128, 8, NCH, 2, K2], F32, name="psw")
            for gsel in range(8):
                nc.tensor.matmul(
                    out=psw[:, gsel].rearrange("p a x b -> p (a x b)"),
                    lhsT=mg_sb[:, gsel, :],
                    rhs=cc[:].rearrange("p a b -> p (a b)"),
                    start=True, stop=True)
            wrapf = wpool.tile([128, 8, NCH, K2], F32, name="wrapf")
            nc.vector.tensor_scalar(wrapf[:], psw[:, :, :, 1, :], 31.0, 0.0,
                                    mult, add)
            wrap = wpool.tile([128, NCH, K2, 8], mybir.dt.int16, name="wrap")
            for wch in range(NCH):
                nc.vector.tensor_tensor(
                    wrap[:, wch].rearrange("q m g -> q g m"),
                    wrapf[:, :, wch], psw[:, :, wch, 0, :], add)

            # ---- 4..7 per-chunk pipeline ----
            psWB = popool.tile([128, NPT], F32, name="psWB")
            rhs = wpool.tile([128, NB, NPT], BF16)
            nc.vector.memset(rhs[64:, NB - 1, :], 0.0)
            out_sb = wpool.tile([COUT, NPT], F32)

            # pieces (ch, klo, khi): the last chunk is split so most of
            # its drain+combine hides under the second piece's desc-gen
            pieces = [(0, 0, 9), (1, 0, 9), (2, 0, 9), (3, 0, 5), (3, 5, 9)]
            samps = [wpool.tile([128, K2, CIN], BF16, name=f"samp{c}")
                     for c in range(NCH)]
            tmps = [wpool.tile([128, K2, CIN], BF16, name=f"tmp{c}")
                    for c in range(NCH)]
            for pidx, (ch, klo, khi) in enumerate(pieces):
                cs = slice(ch * PCH, (ch + 1) * PCH)
                nk = khi - klo
                tc.tile_set_cur_wait(0.009 * pidx)
                ga = wpool.tile([128, nk, 4, CIN], BF16, name=f"ga{pidx}")
                nc.gpsimd.dma_gather(
                    out_ap=ga[:].rearrange("p k t c -> p k (t c)"),
                    in_ap=tbl4.ap(),
                    idxs_ap=wrap[:, ch, klo:khi].rearrange("q m g -> q (m g)"),
                    num_idxs=128 * nk, num_idxs_reg=128 * nk,
                    elem_size=4 * CIN, single_packet=False)

                def bcw(tap):
                    return w4[:, tap, ch, klo:khi][:, :, None].to_broadcast(
                        [128, nk, CIN])

                samp = samps[ch][:, klo:khi]
                tmp = tmps[ch][:, klo:khi]
                nc.vector.tensor_tensor(samp, ga[:, :, 0], bcw(0), mult)
                nc.vector.tensor_tensor(tmp, ga[:, :, 1], bcw(1), mult)
                nc.vector.tensor_tensor(samp, samp, tmp, add)
                nc.vector.tensor_tensor(tmp, ga[:, :, 2], bcw(2), mult)
                nc.vector.tensor_tensor(samp, samp, tmp, add)
                nc.vector.tensor_tensor(tmp, ga[:, :, 3], bcw(3), mult)
                nc.vector.tensor_tensor(samp, samp, tmp, add)

                # transpose samp[q, (k c)] -> rhs[(k c), b, cs], then
                # column-slice matmuls psWB[:, cs] += wwb_b @ s.  For ch3,
                # b-chunks 0,1 (k0-3) are ready after piece 3a and overlap
                # piece 3b's descriptor gen; b2-4 wait for the full samp.
                if khi == 5:
                    brange = range(2)
                elif klo == 5:
                    brange = range(2, NB)
                else:
                    brange = range(NB)
                sf = samps[ch][:].rearrange("p k c -> p (k c)")
                for b in brange:
                    mlo, mhi = 128 * b, min(128 * (b + 1), CIN * K2)
                    pst = tpool.tile([128, 128], BF16, tag="tps")
                    nc.tensor.transpose(
                        pst[:mhi - mlo, :], sf[:, mlo:mhi], identb[:])
                    nc.any.tensor_copy(rhs[:mhi - mlo, b, cs],
                                       pst[:mhi - mlo, :PCH])
                for b in brange:
                    nc.tensor.matmul(
                        out=psWB[:, cs], lhsT=wwb_sb[:, b, :],
                        rhs=rhs[:, b, cs],
                        start=(b == 0), stop=(b == NB - 1))
                if khi < K2:
                    continue
                nc.vector.tensor_tensor(out_sb[:, cs], psWB[:COUT, cs],
                                        xc_sb[:COUT, cs], mult)
                nc.vector.tensor_tensor(out_sb[:, cs], out_sb[:, cs],
                                        psWB[COUT:, cs], add)
                eng = nc.sync if ch % 2 == 0 else nc.scalar
                eng.dma_start(out_d.ap()[:, cs], out_sb[:, cs])

    nc.compile()
    return nc


def _host_inputs(x, w_off, b_off, w_wgt, b_wgt):
    """Build the 8 per-core input dicts (layout/shard prep only)."""
    x = np.asarray(x, dtype=np.float32)
    w_off = np.asarray(w_off, dtype=np.float32)
    b_off = np.asarray(b_off, dtype=np.float32)
    w_wgt = np.asarray(w_wgt, dtype=np.float32)
    b_wgt = np.asarray(b_wgt, dtype=np.float32)

    # wwb [128, 5, 128]: lhsT chunk b = [W~.T | B~.T] on the output axis,
    # k-major contraction order m = k*64 + c.
    perm = np.arange(CIN * K2).reshape(CIN, K2).T.reshape(-1)
    wtp = np.zeros((NB * 128, COUT), dtype=np.float32)
    wtp[:576] = w_wgt.T[perm]
    btp = np.zeros((NB * 128, COUT), dtype=np.float32)
    btp[:576] = b_wgt.reshape(CIN, K2 * COUT).T[perm]
    wwb = np.concatenate([wtp.reshape(NB, 128, COUT),
                          btp.reshape(NB, 128, COUT)], axis=2)
    wwb_b = _to_bf16(np.ascontiguousarray(
        wwb.transpose(1, 0, 2).reshape(128, NB * 128)))

    mg = np.zeros((128, 8, 128), dtype=np.float32)
    q = np.arange(128)
    for gsel in range(8):
        mg[gsel * 16 + (q % 16), gsel, q] = 1.0
    mg = _to_bf16(mg.reshape(128, 8 * 128))

    wofft = np.zeros((128, 2 * K2), dtype=np.float32)
    wofft[:CIN, :K2] = w_off[0::2].T
    wofft[:CIN, K2:] = w_off[1::2].T

    xs = np.linspace(-1.0, 1.0, W).astype(np.float32)
    ys = np.linspace(-1.0, 1.0, H).astype(np.float32)
    kx = np.linspace(-(K - 1) / (W - 1), (K - 1) / (W - 1), K).astype(np.float32)
    ky = np.linspace(-(K - 1) / (H - 1), (K - 1) / (H - 1), K).astype(np.float32)

    in_maps = []
    for c in range(NCORES):
        n, half = divmod(c, 2)
        r0 = HHALF * half
        xn = x[n]

        # 4-tap table on the clipped 31x31 grid; OOB taps are zero.
        pad = np.zeros((CIN, H + 5, W + 5), dtype=np.float32)
        pad[:, 2:2 + H, 2:2 + W] = xn
        t00 = pad[:, 0:31, 0:31]
        t01 = pad[:, 0:31, 1:32]
        t10 = pad[:, 1:32, 0:31]
        t11 = pad[:, 1:32, 1:32]
        tbl = np.stack([t00, t01, t10, t11], axis=0)  # [4, 64, 31, 31]
        tbl = tbl.transpose(2, 3, 0, 1).reshape(TBL_ROWS, 4 * CIN)
        tbl_b = _to_bf16(np.ascontiguousarray(tbl))

        xcpad = np.zeros((128, NPT), dtype=np.float32)
        xcpad[:CIN] = xn.reshape(CIN, H * W)[:, r0 * W:r0 * W + NPT]

        b2 = np.zeros((128, NCH, 2 * K2), dtype=np.float32)
        p_idx = np.arange(PCH)
        for ch in range(NCH):
            gpix = r0 * W + ch * PCH + p_idx
            row, col = gpix // W, gpix % W
            for kk in range(K2):
                kyi, kxi = divmod(kk, K)
                b2[:PCH, ch, kk] = ((xs[col] + kx[kxi] + b_off[2 * kk] + 1.0)
                                    * SC + 2.0)
                b2[:PCH, ch, K2 + kk] = ((ys[row] + ky[kyi] + b_off[2 * kk + 1]
                                          + 1.0) * SC + 2.0)
        b2[PCH:] = SC + 2.0

        in_maps.append({
            "tbl4": tbl_b,
            "xcpad": xcpad,
            "wofft": wofft,
            "base2": b2.reshape(128, NCH * 2 * K2),
            "wwb": wwb_b,
            "mg": mg,
        })
    return in_maps


def _to_bf16(a):
    try:
        import ml_dtypes
        return a.astype(ml_dtypes.bfloat16)
    except ImportError:
        b = a.view(np.uint32)
        rounded = ((b + 0x7FFF + ((b >> 16) & 1)) >> 16).astype(np.uint16)
        return rounded.view(np.uint16)


def get_program():
    if "nc" not in _CACHE:
        _CACHE["nc"] = _build_program()
    return _CACHE["nc"]


def run_cores(in_maps, **kw):
    nc = get_program()
    return run_bass_kernel_spmd(nc, in_maps, core_ids=list(range(NCORES)), **kw)


def assemble(results):
    out = np.zeros((N, COUT, H, W), dtype=np.float32)
    for c in range(NCORES):
        n, half = divmod(c, 2)
        out[n, :, HHALF * half:HHALF * (half + 1), :] = \
            results[c]["out"].reshape(COUT, HHALF, W)
    return out


def kernel(x, w_off, b_off, w_wgt, b_wgt):
    in_maps = _host_inputs(x, w_off, b_off, w_wgt, b_wgt)
    res = run_cores(in_maps)
    return assemble(res.results)


# revision 16
# speedup vs baseline: 1.0171x; 1.0171x over previous
"""Trainium2 Bass kernel for nn_DeformRouting (deformable routing conv), v3.

Strategy (8 cores, data-parallel over N x H-halves):
  core c handles image n = c//2, row-half = c%2 (14 rows x 28 cols = 392 pixels).

Pipeline (baseline 181us -> 72us):
  - 4-tap bf16 table rows (zero-padded 31x31 grid) -> ONE gather index per
    sample; validity masking folded into table zeros.
  - Per-piece pipeline over 5 gather pieces (9/9/9/5/4 k-slots): gather ->
    combine -> transpose -> column-slice matmuls, so only the small last
    piece's tail is exposed behind the serial SWDGE descriptor generation
    (the dominant cost, ~8ns/idx of gpsimd ucode; ~39us total).
  - Index-permutation matmuls run in bf16 on the SPLIT clipped coords
    (cy, cx <= 30 are bf16-exact); row = 31*cy+cx is fused into the int16
    wrap build.  tile_set_cur_wait paces the scheduler so later pieces'
    combines are not hoisted ahead on the in-order DVE queue.
  - k-major contraction (m = k*64+c, host-permuted weights): contiguous
    bf16 combine writes; W~ and B~ stacked in one [128,128] lhsT chunk so 5
    matmuls per column-slice produce [W~@s ; B~@s] together; out =
    psWB[:64]*x + psWB[64:] per the grouped weight-gen algebra.
"""

import numpy as np

import concourse.bass as bass
import concourse.tile as tile
from concourse import bacc, mybir
from concourse.bass_utils import run_bass_kernel_spmd
from concourse.masks import make_identity

N, CIN, COUT, H, W, K = 4, 64, 64, 28, 28, 3
K2 = K * K
NCORES = 8
HHALF = H // 2          # 14 rows per core
NPT = HHALF * W         # 392 points per core
PCH = 98                # points per partition-chunk
NCH = 4                 # chunks
TBL_ROWS = 31 * 31      # 961 4-tap table rows
SC = (W - 1) / 2.0      # 13.5
NI_CH = 128 * K2        # 1152 gathered rows per chunk
NB = 5                  # m-chunks of 128 (576 -> 640)

F32 = mybir.dt.float32
F32R = mybir.dt.float32r
I32 = mybir.dt.int32
BF16 = mybir.dt.bfloat16

_CACHE = {}


def _alu(name):
    return getattr(mybir.AluOpType, name)


def _build_program():
    nc = bacc.Bacc("TRN2", target_bir_lowering=False, debug=False,
                   num_devices=NCORES)

    tbl4 = nc.dram_tensor("tbl4", [TBL_ROWS, 4 * CIN], BF16, kind="ExternalInput")
    xcpad = nc.dram_tensor("xcpad", [128, NPT], F32, kind="ExternalInput")
    wofft = nc.dram_tensor("wofft", [128, 2 * K2], F32, kind="ExternalInput")
    base2 = nc.dram_tensor("base2", [128, NCH * 2 * K2], F32, kind="ExternalInput")
    wwb = nc.dram_tensor("wwb", [128, NB * 128], BF16, kind="ExternalInput")
    mg = nc.dram_tensor("mg", [128, 8 * 128], BF16, kind="ExternalInput")
    out_d = nc.dram_tensor("out", [COUT, NPT], F32, kind="ExternalOutput")

    mult, add, sub = _alu("mult"), _alu("add"), _alu("subtract")
    is_gt = _alu("is_gt")
    amin, amax = _alu("min"), _alu("max")

    with tile.TileContext(nc) as tc:
        with (
            tc.tile_pool(name="const", bufs=1) as cpool,
            tc.tile_pool(name="work", bufs=1) as wpool,
            tc.tile_pool(name="psoff", bufs=1, space="PSUM") as opool,
            tc.tile_pool(name="pst", bufs=2, space="PSUM") as tpool,
            tc.tile_pool(name="pso", bufs=1, space="PSUM") as popool,
        ):
            # ---- inputs, spread across engine DMA queues ----
            xc_sb = cpool.tile([128, NPT], F32)
            nc.sync.dma_start(xc_sb[:], xcpad.ap())
            wofft_sb = cpool.tile([128, 2 * K2], F32)
            nc.scalar.dma_start(wofft_sb[:], wofft.ap())
            base2_sb = cpool.tile([128, NCH, 2 * K2], F32)
            nc.sync.dma_start(base2_sb[:], base2.ap().rearrange(
                "p (a b) -> p a b", a=NCH))
            wwb_sb = cpool.tile([128, NB, 128], BF16)
            nc.scalar.dma_start(wwb_sb[:], wwb.ap().rearrange(
                "p (a b) -> p a b", a=NB))
            mg_sb = cpool.tile([128, 8, 128], BF16)
            nc.sync.dma_start(mg_sb[:], mg.ap().rearrange(
                "p (a b) -> p a b", a=8))
            identb = cpool.tile([128, 128], BF16)
            make_identity(nc, identb[:])

            # ---- 1. offset conv (pad partitions zeroed: garbage would
            # flow into gather indices past the clip) ----
            ps_off = opool.tile([128, NCH, 2 * K2], F32)
            nc.vector.memset(ps_off[:], 0.0)
            for ch in range(NCH):
                nc.tensor.matmul(
                    out=ps_off[:PCH, ch, :],
                    lhsT=xc_sb[:, ch * PCH:(ch + 1) * PCH],
                    rhs=wofft_sb[:],
                    start=True, stop=True,
                )

            # ---- 2. coordinate math on fused x|y tiles [128, NCH, 18] ----
            shp2 = [128, NCH, 2 * K2]
            _cnt = [0]

            def t(shape=shp2, dt=F32):
                _cnt[0] += 1
                return wpool.tile(shape, dt, name=f"ct{_cnt[0]}")

            ic = t()
            nc.vector.scalar_tensor_tensor(ic[:], ps_off[:], SC, base2_sb[:],
                                           mult, add)
            ti = t(dt=I32)
            nc.any.tensor_copy(ti[:], ic[:])
            tf = t()
            nc.any.tensor_copy(tf[:], ti[:])
            g = t()
            nc.vector.tensor_tensor(g[:], tf[:], ic[:], is_gt)
            f0 = t()
            nc.vector.tensor_tensor(f0[:], tf[:], g[:], sub)
            w1 = t()
            nc.vector.tensor_tensor(w1[:], ic[:], f0[:], sub)
            w0 = t()
            nc.vector.tensor_scalar(w0[:], w1[:], -1.0, 1.0, mult, add)
            cc = t(dt=BF16)
            nc.vector.tensor_scalar(cc[:], f0[:], 30.0, 0.0, amin, amax)
            w4 = t([128, 4, NCH, K2], BF16)
            nc.vector.tensor_tensor(w4[:, 0], w0[:, :, K2:], w0[:, :, :K2], mult)
            nc.vector.tensor_tensor(w4[:, 1], w0[:, :, K2:], w1[:, :, :K2], mult)
            nc.vector.tensor_tensor(w4[:, 2], w1[:, :, K2:], w0[:, :, :K2], mult)
            nc.vector.tensor_tensor(w4[:, 3], w1[:, :, K2:], w1[:, :, :K2], mult)

            # ---- 3. idx wrap: permute clipped coords (bf16-exact ints),
            # then fuse row = 31*cy + cx into the int16 wrap build ----
            psw = opool.tile([128, 8, NCH, 2, K2], F32, name="psw")
            for gsel in range(8):
                nc.tensor.matmul(
                    out=psw[:, gsel].rearrange("p a x b -> p (a x b)"),
                    lhsT=mg_sb[:, gsel, :],
                    rhs=cc[:].rearrange("p a b -> p (a b)"),
                    start=True, stop=True)
            wrapf = wpool.tile([128, 8, NCH, K2], F32, name="wrapf")
            nc.vector.tensor_scalar(wrapf[:], psw[:, :, :, 1, :], 31.0, 0.0,
                                    mult, add)
            wrap = wpool.tile([128, NCH, K2, 8], mybir.dt.int16, name="wrap")
            nc.vector.tensor_tensor(
                wrap[:].rearrange("q a m g -> q g a m"), wrapf[:],
                psw[:, :, :, 0, :], add)

            # ---- 4..7 per-chunk pipeline ----
            psWB = popool.tile([128, NPT], F32, name="psWB")
            rhs = wpool.tile([128, NB, NPT], BF16)
            nc.vector.memset(rhs[64:, NB - 1, :], 0.0)
            out_sb = wpool.tile([COUT, NPT], F32)

            # pieces (ch, klo, khi): the last chunk is split so most of
            # its drain+combine hides under the second piece's desc-gen
            pieces = [(0, 0, 9), (1, 0, 9), (2, 0, 9), (3, 0, 5), (3, 5, 9)]
            samps = [wpool.tile([128, K2, CIN], BF16, name=f"samp{c}")
                     for c in range(NCH)]
            tmps = [wpool.tile([128, K2, CIN], BF16, name=f"tmp{c}")
                    for c in range(NCH)]
            for pidx, (ch, klo, khi) in enumerate(pieces):
                cs = slice(ch * PCH, (ch + 1) * PCH)
                nk = khi - klo
                tc.tile_set_cur_wait(0.009 * pidx)
                ga = wpool.tile([128, nk, 4, CIN], BF16, name=f"ga{pidx}")
                nc.gpsimd.dma_gather(
                    out_ap=ga[:].rearrange("p k t c -> p k (t c)"),
                    in_ap=tbl4.ap(),
                    idxs_ap=wrap[:, ch, klo:khi].rearrange("q m g -> q (m g)"),
                    num_idxs=128 * nk, num_idxs_reg=128 * nk,
                    elem_size=4 * CIN, single_packet=False)

                def bcw(tap):
                    return w4[:, tap, ch, klo:khi][:, :, None].to_broadcast(
                        [128, nk, CIN])

                samp = samps[ch][:, klo:khi]
                tmp = tmps[ch][:, klo:khi]
                nc.vector.tensor_tensor(samp, ga[:, :, 0], bcw(0), mult)
                nc.vector.tensor_tensor(tmp, ga[:, :, 1], bcw(1), mult)
                nc.vector.tensor_tensor(samp, samp, tmp, add)
                nc.vector.tensor_tensor(tmp, ga[:, :, 2], bcw(2), mult)
                nc.vector.tensor_tensor(samp, samp, tmp, add)
                nc.vector.tensor_tensor(tmp, ga[:, :, 3], bcw(3), mult)
                nc.vector.tensor_tensor(samp, samp, tmp, add)

                if khi < K2:
                    continue
                # transpose samp[q, (k c)] -> rhs[(k c), b, cs]
                sf = samps[ch][:].rearrange("p k c -> p (k c)")
                for b in range(NB):
                    mlo, mhi = 128 * b, min(128 * (b + 1), CIN * K2)
                    pst = tpool.tile([128, 128], BF16, tag="tps")
                    nc.tensor.transpose(
                        pst[:mhi - mlo, :], sf[:, mlo:mhi], identb[:])
                    nc.any.tensor_copy(rhs[:mhi - mlo, b, cs],
                                       pst[:mhi - mlo, :PCH])

                # column-slice matmuls: psWB[:, cs] = [W~ ; B~] @ s_ch
                for b in range(NB):
                    nc.tensor.matmul(
                        out=psWB[:, cs], lhsT=wwb_sb[:, b, :],
                        rhs=rhs[:, b, cs],
                        start=(b == 0), stop=(b == NB - 1))
                nc.vector.tensor_tensor(out_sb[:, cs], psWB[:COUT, cs],
                                        xc_sb[:COUT, cs], mult)
                nc.vector.tensor_tensor(out_sb[:, cs], out_sb[:, cs],
                                        psWB[COUT:, cs], add)
                eng = nc.sync if ch % 2 == 0 else nc.scalar
                eng.dma_start(out_d.ap()[:, cs], out_sb[:, cs])

    nc.compile()
    return nc


def _host_inputs(x, w_off, b_off, w_wgt, b_wgt):
    """Build the 8 per-core input dicts (layout/shard prep only)."""
    x = np.asarray(x, dtype=np.float32)
    w_off = np.asarray(w_off, dtype=np.float32)
    b_off = np.asarray(b_off, dtype=np.float32)
    w_wgt = np.asarray(w_wgt, dtype=np.float32)
    b_wgt = np.asarray(b_wgt, dtype=np.float32)

    # wwb [128, 5, 128]: lhsT chunk b = [W~.T | B~.T] on the output axis,
    # k-major contraction order m = k*64 + c.
    perm = np.arange(CIN * K2).reshape(CIN, K2).T.reshape(-1)
    wtp = np.zeros((NB * 128, COUT), dtype=np.float32)
    wtp[:576] = w_wgt.T[perm]
    btp = np.zeros((NB * 128, COUT), dtype=np.float32)
    btp[:576] = b_wgt.reshape(CIN, K2 * COUT).T[perm]
    wwb = np.concatenate([wtp.reshape(NB, 128, COUT),
                          btp.reshape(NB, 128, COUT)], axis=2)
    wwb_b = _to_bf16(np.ascontiguousarray(
        wwb.transpose(1, 0, 2).reshape(128, NB * 128)))

    mg = np.zeros((128, 8, 128), dtype=np.float32)
    q = np.arange(128)
    for gsel in range(8):
        mg[gsel * 16 + (q % 16), gsel, q] = 1.0
    mg = _to_bf16(mg.reshape(128, 8 * 128))

    wofft = np.zeros((128, 2 * K2), dtype=np.float32)
    wofft[:CIN, :K2] = w_off[0::2].T
    wofft[:CIN, K2:] = w_off[1::2].T

    xs = np.linspace(-1.0, 1.0, W).astype(np.float32)
    ys = np.linspace(-1.0, 1.0, H).astype(np.float32)
    kx = np.linspace(-(K - 1) / (W - 1), (K - 1) / (W - 1), K).astype(np.float32)
    ky = np.linspace(-(K - 1) / (H - 1), (K - 1) / (H - 1), K).astype(np.float32)

    in_maps = []
    for c in range(NCORES):
        n, half = divmod(c, 2)
        r0 = HHALF * half
        xn = x[n]

        # 4-tap table on the clipped 31x31 grid; OOB taps are zero.
        pad = np.zeros((CIN, H + 5, W + 5), dtype=np.float32)
        pad[:, 2:2 + H, 2:2 + W] = xn
        t00 = pad[:, 0:31, 0:31]
        t01 = pad[:, 0:31, 1:32]
        t10 = pad[:, 1:32, 0:31]
        t11 = pad[:, 1:32, 1:32]
        tbl = np.stack([t00, t01, t10, t11], axis=0)  # [4, 64, 31, 31]
        tbl = tbl.transpose(2, 3, 0, 1).reshape(TBL_ROWS, 4 * CIN)
        tbl_b = _to_bf16(np.ascontiguousarray(tbl))

        xcpad = np.zeros((128, NPT), dtype=np.float32)
        xcpad[:CIN] = xn.reshape(CIN, H * W)[:, r0 * W:r0 * W + NPT]

        b2 = np.zeros((128, NCH, 2 * K2), dtype=np.float32)
        p_idx = np.arange(PCH)
        for ch in range(NCH):
            gpix = r0 * W + ch * PCH + p_idx
            row, col = gpix // W, gpix % W
            for kk in range(K2):
                kyi, kxi = divmod(kk, K)
                b2[:PCH, ch, kk] = ((xs[col] + kx[kxi] + b_off[2 * kk] + 1.0)
                                    * SC + 2.0)
                b2[:PCH, ch, K2 + kk] = ((ys[row] + ky[kyi] + b_off[2 * kk + 1]
                                          + 1.0) * SC + 2.0)
        b2[PCH:] = SC + 2.0

        in_maps.append({
            "tbl4": tbl_b,
            "xcpad": xcpad,
            "wofft": wofft,
            "base2": b2.reshape(128, NCH * 2 * K2),
            "wwb": wwb_b,
            "mg": mg,
        })
    return in_maps


def _to_bf16(a):
    try:
        import ml_dtypes
        return a.astype(ml_dtypes.bfloat16)
    except ImportError:
        b = a.view(np.uint32)
        rounded = ((b + 0x7FFF + ((b >> 16) & 1)) >> 16).astype(np.uint16)
        return rounded.view(np.uint16)


def get_program():
    if "nc" not in _CACHE:
        _CACHE["nc"] = _build_program()
    return _CACHE["nc"]


def run_cores(in_maps, **kw):
    nc = get_program()
    return run_bass_kernel_spmd(nc, in_maps, core_ids=list(range(NCORES)), **kw)


def assemble(results):
    out = np.zeros((N, COUT, H, W), dtype=np.float32)
    for c in range(NCORES):
        n, half = divmod(c, 2)
        out[n, :, HHALF * half:HHALF * (half + 1), :] = \
            results[c]["out"].reshape(COUT, HHALF, W)
    return out


def kernel(x, w_off, b_off, w_wgt, b_wgt):
    in_maps = _host_inputs(x, w_off, b_off, w_wgt, b_wgt)
    res = run_cores(in_maps)
    return assemble(res.results)
